# revision 1
# baseline (speedup 1.0000x reference)
"""MiniMax-Text-01 Lightning Attention on 8 Trainium2 NeuronCores (Bass/Tile).

Sharding: data-parallel over batch (2) x tensor-parallel over heads (16 -> 4
groups of 4 heads). Core c handles b = c//4, heads [4*(c%4), 4*(c%4)+4).

Single fused per-chunk pipeline (BLK=256 tokens/chunk, attention scanned in
two 128-token sub-blocks): qkv+gate projections (bf16 matmuls), lightning
attention scan, RMS-norm partials (sum-of-squares), gated out-projection to a
partial y [4096, 2048] (bf16). Host combines: y_b = sum_c y_c * rsqrt(sum_c
ssq_c / 2048 + eps) (the norm scale factors out of the linear out projection;
norm_w is folded into w_out on the host).

All matmul operands are bf16 (1 cyc/row on the PE; fp32r pays 4x on the
128-wide kv updates and 1.5x on transposes). Per-sub-block attention matmul
outputs are packed as column slices of single [128,512] PSUM bank tiles so the
DVE/ACT drains run as single wide ops.
"""
import numpy as np
import ml_dtypes
from contextlib import ExitStack

import concourse.bass as bass
import concourse.tile as tile
from concourse import mybir, bacc
from concourse.bass_utils import run_bass_kernel_spmd

B, N, H = 2, 4096, 2048
NH, HD = 16, 128
BLK = 256
SUB = 128             # attention scan block
EPS = 1e-6
NCORES = 8
HPC = 4               # heads per core
OC = HPC * HD         # 512 per-core channels
NCHUNK = N // BLK     # 16 chunks of 256 tokens
KT = H // 128         # 16 contraction tiles
F32 = mybir.dt.float32
BF16 = mybir.dt.bfloat16
AF = mybir.ActivationFunctionType
ALU = mybir.AluOpType
BF = ml_dtypes.bfloat16


def build_nc(repeats=1):
    nc = bacc.Bacc("TRN2", target_bir_lowering=False, debug=False,
                   num_devices=NCORES)
    x_d = nc.dram_tensor("x_sb", [128, NCHUNK, KT * BLK], BF16, kind="ExternalInput")
    wq_d = nc.dram_tensor("wq_sb", [128, KT * OC], BF16, kind="ExternalInput")
    wk_d = nc.dram_tensor("wk_sb", [128, KT * OC], BF16, kind="ExternalInput")
    wv_d = nc.dram_tensor("wv_sb", [128, KT * OC], BF16, kind="ExternalInput")
    wg_d = nc.dram_tensor("wg_sb", [128, KT * OC], BF16, kind="ExternalInput")
    wo_d = nc.dram_tensor("wo_sb", [128, HPC * H], BF16, kind="ExternalInput")
    dT_d = nc.dram_tensor("decayT", [128, HPC * SUB], F32, kind="ExternalInput")
    qd_d = nc.dram_tensor("qdec", [128, HPC * SUB], F32, kind="ExternalInput")
    kdc_d = nc.dram_tensor("kdecc", [128, HPC], F32, kind="ExternalInput")
    bd_d = nc.dram_tensor("bdec", [128, HPC], F32, kind="ExternalInput")
    ones_d = nc.dram_tensor("ones_sb", [128, 1], BF16, kind="ExternalInput")
    id_d = nc.dram_tensor("ident_sb", [128, 128], BF16, kind="ExternalInput")
    y_d = nc.dram_tensor("y", [N, H], BF16, kind="ExternalOutput")
    ssq_d = nc.dram_tensor("ssq", [1, N], F32, kind="ExternalOutput")

    with tile.TileContext(nc) as tc, ExitStack() as ctx:
        constp = ctx.enter_context(tc.tile_pool(name="const", bufs=1))
        ident = constp.tile([128, 128], BF16)
        nc.sync.dma_start(ident[:], id_d[:])
        dT = constp.tile([128, HPC * SUB], F32)
        nc.sync.dma_start(dT[:], dT_d[:])
        qd = constp.tile([128, HPC * SUB], F32)
        nc.sync.dma_start(qd[:], qd_d[:])
        kdc = constp.tile([128, HPC], F32)
        nc.sync.dma_start(kdc[:], kdc_d[:])
        bdc = constp.tile([128, HPC], F32)
        nc.sync.dma_start(bdc[:], bd_d[:])
        ones = constp.tile([128, 1], BF16)
        nc.sync.dma_start(ones[:], ones_d[:])

        with (
            tc.tile_pool(name="wp", bufs=1) as wp,
            tc.tile_pool(name="xp", bufs=2) as xp,
            tc.tile_pool(name="qkp", bufs=2) as qkp,
            tc.tile_pool(name="gp", bufs=2) as gp,
            tc.tile_pool(name="vp", bufs=4) as vp,
            tc.tile_pool(name="qkmp", bufs=3) as qkmp,
            tc.tile_pool(name="ktokp", bufs=3) as ktokp,
            tc.tile_pool(name="qtdp", bufs=3) as qtdp,
            tc.tile_pool(name="kvp", bufs=3) as kvp,
            tc.tile_pool(name="zp", bufs=4) as zp,
            tc.tile_pool(name="otp", bufs=3) as otp,
            tc.tile_pool(name="sqp", bufs=3) as sqp,
            tc.tile_pool(name="ssqp", bufs=4) as ssqp,
            tc.tile_pool(name="ysp", bufs=4) as ysp,
            tc.tile_pool(name="pq", bufs=2, space="PSUM") as pqp,
            tc.tile_pool(name="py", bufs=2, space="PSUM") as pyp,
            tc.tile_pool(name="pa", bufs=4, space="PSUM") as pap,
        ):
            for rep in range(repeats):
                _body(nc, tc, rep == 0, ident, dT, qd, kdc, bdc, ones,
                      x_d, wq_d, wk_d, wv_d, wg_d, wo_d, y_d, ssq_d,
                      wp, xp, qkp, gp, vp, qkmp, ktokp, qtdp, kvp, zp, otp,
                      sqp, ssqp, ysp, pqp, pyp, pap)

    nc.finalize()
    return nc


def _body(nc, tc, first, ident, dT, qd, kdc, bdc, ones,
          x_d, wq_d, wk_d, wv_d, wg_d, wo_d, y_d, ssq_d,
          wp, xp, qkp, gp, vp, qkmp, ktokp, qtdp, kvp, zp, otp,
          sqp, ssqp, ysp, pqp, pyp, pap):
    # first x chunk, then weights in consumption order
    xt0 = xp.tile([128, KT * BLK], BF16, tag="xt")
    nc.sync.dma_start(xt0[:], x_d[:, 0, :])
    wq = wp.tile([128, KT * OC], BF16, tag="wq")
    wk = wp.tile([128, KT * OC], BF16, tag="wk")
    wv = wp.tile([128, KT * OC], BF16, tag="wv")
    wg = wp.tile([128, KT * OC], BF16, tag="wg")
    wo = wp.tile([128, HPC * H], BF16, tag="wo")
    if first:
        # kt-sliced so the first q-proj matmul can start after slice 0
        for kt in range(KT):
            nc.sync.dma_start(wq[:, kt * OC:(kt + 1) * OC],
                              wq_d[:, kt * OC:(kt + 1) * OC])
    else:
        nc.sync.dma_start(wq[:], wq_d[:])
    nc.sync.dma_start(wk[:], wk_d[:])
    nc.sync.dma_start(wv[:], wv_d[:])
    nc.sync.dma_start(wg[:], wg_d[:])
    nc.sync.dma_start(wo[:], wo_d[:])

    if first:
        # keep the PE busy during the initial DMA wait so HAM is warm
        pwarm = pap.tile([128, 512], BF16, tag="pa")
        for _w in range(40):
            nc.tensor.transpose(pwarm[:, 0:128], ident[:], ident[:])

    kv_cur = None
    xt = xt0

    for c in range(NCHUNK):
        t0 = c * BLK
        if c + 1 < NCHUNK:
            xt_next = xp.tile([128, KT * BLK], BF16, tag="xt")
            nc.sync.dma_start(xt_next[:], x_d[:, c + 1, :])
        else:
            xt_next = None

        # ---- q, k projections (channel-major [d, tok]) -> silu -> bf16
        qT = qkp.tile([128, HPC * BLK], BF16, tag="qT")
        kT = qkp.tile([128, HPC * BLK], BF16, tag="kT")
        for (wmat, dst) in ((wq, qT), (wk, kT)):
            for h in range(HPC):
                pq = pqp.tile([128, BLK], F32, tag="pq")
                for kt in range(KT):
                    nc.tensor.matmul(
                        pq[:],
                        lhsT=wmat[:, kt * OC + h * HD: kt * OC + (h + 1) * HD],
                        rhs=xt[:, kt * BLK:(kt + 1) * BLK],
                        start=(kt == 0), stop=(kt == KT - 1),
                    )
                nc.scalar.activation(dst[:, h * BLK:(h + 1) * BLK], pq[:], AF.Silu)

        # ---- v projection (token-major [tok, ch]) -> silu -> bf16
        vt = []
        for s in range(2):
            pv = pyp.tile([128, OC], F32, tag="py")
            for kt in range(KT):
                nc.tensor.matmul(
                    pv[:],
                    lhsT=xt[:, kt * BLK + s * SUB: kt * BLK + s * SUB + SUB],
                    rhs=wv[:, kt * OC:(kt + 1) * OC],
                    start=(kt == 0), stop=(kt == KT - 1),
                )
            v = vp.tile([128, OC], BF16, tag="vt")
            nc.scalar.activation(v[:], pv[:], AF.Silu)
            vt.append(v)

        # ---- q * qdec for the inter-block term (SBUF-only -> pool engine)
        qT3 = qT[:].rearrange("p (h t) -> p h t", h=HPC)
        qd3 = qd[:].rearrange("p (h t) -> p h t", h=HPC)
        qTd = []
        for s in range(2):
            if c == 0 and s == 0:
                qTd.append(None)
                continue
            qtd = qtdp.tile([128, OC], BF16, tag="qtd")
            qtd3 = qtd[:].rearrange("p (h t) -> p h t", h=HPC)
            nc.gpsimd.tensor_mul(qtd3, qT3[:, :, s * SUB:(s + 1) * SUB], qd3)
            qTd.append(qtd)

        def attn_front(s):
            """qk + k-transpose matmuls for sub-block s (+ DVE/ACT drains)."""
            ts = s * SUB
            pqk = pap.tile([128, 4 * SUB], F32, tag="pa")
            for h in range(HPC):
                nc.tensor.matmul(
                    pqk[:, h * SUB:(h + 1) * SUB],
                    lhsT=kT[:, h * BLK + ts: h * BLK + ts + SUB],
                    rhs=qT[:, h * BLK + ts: h * BLK + ts + SUB],
                    start=True, stop=True,
                )
            qkm = qkmp.tile([128, 4 * SUB], BF16, tag="qkm")
            nc.vector.tensor_mul(qkm[:], pqk[:], dT[:])

            ptr = pap.tile([128, 4 * SUB], BF16, tag="pa")
            for h in range(HPC):
                nc.tensor.transpose(
                    ptr[:, h * SUB:(h + 1) * SUB],
                    kT[:, h * BLK + ts: h * BLK + ts + SUB], ident[:])
            ktok = ktokp.tile([128, 4 * SUB], BF16, tag="ktok")
            for h in range(HPC):
                nc.scalar.activation(
                    ktok[:, h * SUB:(h + 1) * SUB],
                    ptr[:, h * SUB:(h + 1) * SUB],
                    AF.Copy, scale=kdc[:, h:h + 1])
            return qkm, ktok

        def attn_back(s, qkm, ktok):
            """o, kv-update matmuls; oT/sq drains. Returns (oT, sq)."""
            nonlocal kv_cur
            po = pap.tile([128, 4 * SUB], F32, tag="pa")
            for h in range(HPC):
                nc.tensor.matmul(
                    po[:, h * SUB:(h + 1) * SUB],
                    lhsT=vt[s][:, h * HD:(h + 1) * HD],
                    rhs=qkm[:, h * SUB:(h + 1) * SUB],
                    start=True, stop=(c == 0 and s == 0),
                )
                if not (c == 0 and s == 0):
                    nc.tensor.matmul(
                        po[:, h * SUB:(h + 1) * SUB],
                        lhsT=kv_cur[:, h * HD:(h + 1) * HD],
                        rhs=qTd[s][:, h * SUB:(h + 1) * SUB],
                        start=False, stop=True,
                    )
            oT = otp.tile([128, 4 * SUB], BF16, tag="oT")
            nc.scalar.activation(oT[:], po[:], AF.Copy)
            sq = sqp.tile([128, 4 * SUB], BF16, tag="sq")
            nc.gpsimd.tensor_mul(sq[:], oT[:], oT[:])

            pkv = pap.tile([128, 4 * SUB], F32, tag="pa")
            for h in range(HPC):
                nc.tensor.matmul(
                    pkv[:, h * HD:(h + 1) * HD],
                    lhsT=ktok[:, h * SUB:(h + 1) * SUB],
                    rhs=vt[s][:, h * HD:(h + 1) * HD],
                    start=True, stop=True,
                )
            kv_new = kvp.tile([128, OC], BF16, tag="kv")
            if c == 0 and s == 0:
                nc.vector.tensor_copy(kv_new[:], pkv[:])
            else:
                for h in range(HPC):
                    nc.vector.scalar_tensor_tensor(
                        out=kv_new[:, h * HD:(h + 1) * HD],
                        in0=kv_cur[:, h * HD:(h + 1) * HD],
                        scalar=bdc[:, h:h + 1],
                        in1=pkv[:, h * HD:(h + 1) * HD],
                        op0=ALU.mult, op1=ALU.add)
            kv_cur = kv_new
            return oT, sq

        # sub-block 0, then sub-block 1's front matmuls, then the gate
        # projection as PE cover while DVE/ACT drain the scan chain
        qkm0, ktok0 = attn_front(0)
        oT0, sq0 = attn_back(0, qkm0, ktok0)
        qkm1, ktok1 = attn_front(1)

        gT = gp.tile([128, 2 * OC], BF16, tag="gT")
        gT3 = gT[:].rearrange("p (s ht) -> p s ht", s=2)
        for h in range(HPC):
            pg = pqp.tile([128, BLK], F32, tag="pq")
            for kt in range(KT):
                nc.tensor.matmul(
                    pg[:],
                    lhsT=wg[:, kt * OC + h * HD: kt * OC + (h + 1) * HD],
                    rhs=xt[:, kt * BLK:(kt + 1) * BLK],
                    start=(kt == 0), stop=(kt == KT - 1),
                )
            pg3 = pg[:].rearrange("p (s t) -> p s t", s=2)
            nc.scalar.activation(
                gT3[:, :, h * SUB:(h + 1) * SUB], pg3, AF.Sigmoid)

        z0 = zp.tile([128, 4 * SUB], BF16, tag="z")
        nc.gpsimd.tensor_mul(z0[:], oT0[:], gT[:, 0:OC])

        oT1, sq1 = attn_back(1, qkm1, ktok1)
        z1 = zp.tile([128, 4 * SUB], BF16, tag="z")
        nc.gpsimd.tensor_mul(z1[:], oT1[:], gT[:, OC:2 * OC])

        # ---- rms-norm partials + out projection (partial y over full H)
        for s, (z, sq) in enumerate(((z0, sq0), (z1, sq1))):
            pssq = pqp.tile([1, SUB], F32, tag="pq")
            for h in range(HPC):
                nc.tensor.matmul(pssq[:], lhsT=ones[:],
                                 rhs=sq[:, h * SUB:(h + 1) * SUB],
                                 start=(h == 0), stop=(h == HPC - 1))
            ssq_s = ssqp.tile([1, SUB], F32, tag="ssqs")
            nc.vector.tensor_copy(ssq_s[:], pssq[:])
            nc.sync.dma_start(ssq_d[0:1, t0 + s * SUB: t0 + (s + 1) * SUB],
                              ssq_s[:])
            for fb in range(4):
                py = pyp.tile([128, 512], F32, tag="py")
                for h in range(HPC):
                    nc.tensor.matmul(
                        py[:],
                        lhsT=z[:, h * SUB:(h + 1) * SUB],
                        rhs=wo[:, h * H + fb * 512: h * H + (fb + 1) * 512],
                        start=(h == 0), stop=(h == HPC - 1),
                    )
                y_s = ysp.tile([128, 512], BF16, tag="ys")
                if fb % 2 == 0:
                    nc.vector.tensor_copy(y_s[:], py[:])
                else:
                    nc.scalar.activation(y_s[:], py[:], AF.Copy)
                nc.sync.dma_start(
                    y_d[t0 + s * SUB: t0 + (s + 1) * SUB,
                        fb * 512:(fb + 1) * 512],
                    y_s[:],
                )

        xt = xt_next


_NC_CACHE = {}


def get_nc():
    if "nc" not in _NC_CACHE:
        _NC_CACHE["nc"] = build_nc()
    return _NC_CACHE["nc"]


def _prep_core_inputs(hidden_states, slope_rate, w_qkv, w_gate, w_out, norm_w):
    """Returns list of 8 in_map dicts."""
    x = np.asarray(hidden_states, dtype=np.float32)
    slopes = np.asarray(slope_rate, dtype=np.float32).reshape(NH)
    w_qkv = np.asarray(w_qkv, dtype=np.float32)
    w_gate = np.asarray(w_gate, dtype=np.float32)
    w_out = np.asarray(w_out, dtype=np.float32)
    norm_w = np.asarray(norm_w, dtype=np.float32)

    def to_sb(wT):  # [2048, F] -> [128, KT*F] bf16
        f = wT.shape[1]
        return np.ascontiguousarray(
            wT.reshape(KT, 128, f).transpose(1, 0, 2).reshape(128, KT * f)
        ).astype(BF)

    x_sb = []
    for b in range(B):
        xT = x[b].T  # [2048 f, 4096 n]
        # [128, NCHUNK, KT*BLK]: [p, c, kt*256+t] = xT[kt*128+p, c*256+t]
        xs = xT.reshape(KT, 128, NCHUNK, BLK).transpose(1, 2, 0, 3)
        x_sb.append(np.ascontiguousarray(
            xs.reshape(128, NCHUNK, KT * BLK)).astype(BF))

    arr = np.arange(1, SUB + 1, dtype=np.float32)
    in_maps = []
    for core in range(NCORES):
        b, hg = divmod(core, HPC)
        heads = [HPC * hg + j for j in range(HPC)]
        wq = np.concatenate([w_qkv[h * 384: h * 384 + 128] for h in heads], 0)
        wk = np.concatenate([w_qkv[h * 384 + 128: h * 384 + 256] for h in heads], 0)
        wv = np.concatenate([w_qkv[h * 384 + 256: h * 384 + 384] for h in heads], 0)
        wg = w_gate[hg * OC:(hg + 1) * OC]
        wo = w_out[:, hg * OC:(hg + 1) * OC] * norm_w[None, hg * OC:(hg + 1) * OC]
        # wo: [2048 f, 512 o] -> [128 p(o), 4 h, 2048 f]
        wo_sb = np.ascontiguousarray(
            wo.T.reshape(HPC, 128, H).transpose(1, 0, 2).reshape(128, HPC * H)
        ).astype(BF)

        dTc = np.zeros((128, HPC * SUB), dtype=np.float32)
        qdec = np.zeros((128, HPC * SUB), dtype=np.float32)
        kdcc = np.zeros((128, HPC), dtype=np.float32)
        bdec = np.zeros((128, HPC), dtype=np.float32)
        for j, h in enumerate(heads):
            s = slopes[h]
            idx = arr[:, None] - arr[None, :]  # [m, n]
            full = np.where(idx >= 0, np.exp(-s * np.maximum(idx, 0.0)),
                            0.0).astype(np.float32)
            # decayT[n, m] layout: partition n (k-token), col m (q-token)
            dTc[:, j * SUB:(j + 1) * SUB] = full.T
            kdcc[:, j] = np.exp(-s * (SUB - arr))
            qdec[:, j * SUB:(j + 1) * SUB] = np.exp(-s * arr)[None, :]
            bdec[:, j] = np.exp(-s * np.float32(SUB))

        in_maps.append({
            "ones_sb": np.ones((128, 1), dtype=BF),
            "ident_sb": np.eye(128, dtype=np.float32).astype(BF),
            "x_sb": x_sb[b],
            "wq_sb": to_sb(np.ascontiguousarray(wq.T)),
            "wk_sb": to_sb(np.ascontiguousarray(wk.T)),
            "wv_sb": to_sb(np.ascontiguousarray(wv.T)),
            "wg_sb": to_sb(np.ascontiguousarray(wg.T)),
            "wo_sb": wo_sb,
            "decayT": dTc,
            "qdec": qdec,
            "kdecc": kdcc,
            "bdec": bdec,
        })
    return in_maps


def _assemble(results):
    out = np.zeros((B, N, H), dtype=np.float32)
    for b in range(B):
        ys = [np.asarray(results[HPC * b + g]["y"], dtype=np.float32)
              for g in range(HPC)]
        ssqs = [results[HPC * b + g]["ssq"].reshape(N).astype(np.float32)
                for g in range(HPC)]
        y_sum = ys[0] + ys[1] + ys[2] + ys[3]
        ssq = ssqs[0] + ssqs[1] + ssqs[2] + ssqs[3]
        rfac = 1.0 / np.sqrt(ssq / np.float32(NH * HD) + np.float32(EPS))
        out[b] = y_sum * rfac[:, None].astype(np.float32)
    return out


def kernel(hidden_states, slope_rate, w_qkv, w_gate, w_out, norm_w):
    nc = get_nc()
    in_maps = _prep_core_inputs(hidden_states, slope_rate, w_qkv, w_gate,
                                w_out, norm_w)
    res = run_bass_kernel_spmd(nc, in_maps, core_ids=list(range(NCORES)))
    return _assemble(res.results)



# revision 3
# speedup vs baseline: 1.3347x; 1.3347x over previous
"""MiniMax-Text-01 Lightning Attention on 8 Trainium2 NeuronCores (Bass/Tile).

Sharding: data-parallel over batch (2) x tensor-parallel over heads (16 -> 4
groups of 4 heads). Core c handles b = c//4, heads [4*(c%4), 4*(c%4)+4).

Software-pipelined per-chunk schedule (BLK=256 tokens/chunk, attention scanned
in two 128-token sub-blocks): the qkv+gate projections for chunk c+1 are
interleaved (at per-head matmul-chain granularity) into the attention scan and
gated out-projection of chunk c, so the PE never waits on ACT/DVE/Pool drains.

The gate projection runs as fp8(e4m3) DoubleRow matmuls (0.5 cyc/row, 256-deep
contraction per pass) - sigmoid's <=1/4 slope damps the quantization noise.
Everything else is bf16 (1 cyc/row). Host combines partial outputs:
y_b = sum_c y_c * rsqrt(sum_c ssq_c / 2048 + eps); norm_w folded into w_out.
"""
import numpy as np
import ml_dtypes
from contextlib import ExitStack

import concourse.bass as bass
import concourse.tile as tile
from concourse import mybir, bacc
from concourse.bass_utils import run_bass_kernel_spmd

B, N, H = 2, 4096, 2048
NH, HD = 16, 128
BLK = 256
SUB = 128             # attention scan block
EPS = 1e-6
NCORES = 8
HPC = 4               # heads per core
OC = HPC * HD         # 512 per-core channels
NCHUNK = N // BLK     # 16 chunks of 256 tokens
KT = H // 128         # 16 contraction tiles
KT8 = KT // 2         # 8 double-row contraction tiles (256 deep)
SW_G = 32.0           # gate-weight fp8 scale (pow2: exact descale)
F32 = mybir.dt.float32
BF16 = mybir.dt.bfloat16
FP8 = mybir.dt.float8e4
F8NP = mybir.dt.np(mybir.dt.float8e4)
AF = mybir.ActivationFunctionType
ALU = mybir.AluOpType
DR = mybir.MatmulPerfMode.DoubleRow
BF = ml_dtypes.bfloat16


def build_nc(repeats=1):
    nc = bacc.Bacc("TRN2", target_bir_lowering=False, debug=False,
                   num_devices=NCORES)
    x_d = nc.dram_tensor("x_sb", [128, NCHUNK, KT * BLK], BF16, kind="ExternalInput")
    x8_d = nc.dram_tensor("x8_sb", [128, NCHUNK, KT * BLK], FP8, kind="ExternalInput")
    wq_d = nc.dram_tensor("wq_sb", [128, KT * OC], BF16, kind="ExternalInput")
    wk_d = nc.dram_tensor("wk_sb", [128, KT * OC], BF16, kind="ExternalInput")
    wv_d = nc.dram_tensor("wv_sb", [128, KT * OC], BF16, kind="ExternalInput")
    wg8_d = nc.dram_tensor("wg8_sb", [128, KT8 * 2 * OC], FP8, kind="ExternalInput")
    wo_d = nc.dram_tensor("wo_sb", [128, HPC * H], BF16, kind="ExternalInput")
    dT_d = nc.dram_tensor("decayT", [128, HPC * SUB], F32, kind="ExternalInput")
    qd_d = nc.dram_tensor("qdec", [128, HPC * SUB], F32, kind="ExternalInput")
    kdc_d = nc.dram_tensor("kdecc", [128, HPC], F32, kind="ExternalInput")
    bd_d = nc.dram_tensor("bdec", [128, HPC], F32, kind="ExternalInput")
    ones_d = nc.dram_tensor("ones_sb", [128, 1], BF16, kind="ExternalInput")
    id_d = nc.dram_tensor("ident_sb", [128, 128], BF16, kind="ExternalInput")
    y_d = nc.dram_tensor("y", [N, H], BF16, kind="ExternalOutput")
    ssq_d = nc.dram_tensor("ssq", [1, N], F32, kind="ExternalOutput")

    with tile.TileContext(nc) as tc, ExitStack() as ctx:
        constp = ctx.enter_context(tc.tile_pool(name="const", bufs=1))
        ident = constp.tile([128, 128], BF16)
        nc.sync.dma_start(ident[:], id_d[:])
        dT = constp.tile([128, HPC * SUB], F32)
        nc.sync.dma_start(dT[:], dT_d[:])
        qd = constp.tile([128, HPC * SUB], F32)
        nc.sync.dma_start(qd[:], qd_d[:])
        kdc = constp.tile([128, HPC], F32)
        nc.sync.dma_start(kdc[:], kdc_d[:])
        bdc = constp.tile([128, HPC], F32)
        nc.sync.dma_start(bdc[:], bd_d[:])
        ones = constp.tile([128, 1], BF16)
        nc.sync.dma_start(ones[:], ones_d[:])

        pool_spec = [("wp", 1, None), ("xp", 3, None), ("x8p", 3, None),
                     ("qkp", 2, None), ("gp", 2, None), ("vp", 4, None),
                     ("sigp", 3, None), ("qkmp", 3, None), ("ktokp", 3, None),
                     ("qtdp", 4, None), ("kvp", 3, None), ("zp", 4, None),
                     ("otp", 3, None), ("sqp", 3, None), ("ssqp", 4, None),
                     ("ysp", 4, None), ("pqp", 2, "PSUM"), ("pyp", 2, "PSUM"),
                     ("pap", 4, "PSUM")]
        pools = {}
        for nm, bufs, space in pool_spec:
            kw = {"space": space} if space else {}
            pools[nm] = ctx.enter_context(tc.tile_pool(name=nm, bufs=bufs, **kw))
        consts = dict(ident=ident, dT=dT, qd=qd, kdc=kdc, bdc=bdc,
                      ones=ones)
        dram = dict(x_d=x_d, x8_d=x8_d, wq_d=wq_d, wk_d=wk_d, wv_d=wv_d,
                    wg8_d=wg8_d, wo_d=wo_d, y_d=y_d, ssq_d=ssq_d)
        for rep in range(repeats):
            _emit_rep(nc, rep == 0, pools, consts, dram)

    nc.finalize()
    return nc


def _emit_rep(nc, first, P, C, D):
    st = {}

    def dma_x(c):
        if c >= NCHUNK:
            return
        t = P["xp"].tile([128, KT * BLK], BF16, tag="xt")
        nc.sync.dma_start(t[:], D["x_d"][:, c, :])
        st[("x", c)] = t
        t8 = P["x8p"].tile([128, KT * BLK], FP8, tag="x8")
        nc.sync.dma_start(t8[:], D["x8_d"][:, c, :])
        st[("x8", c)] = t8

    # ---- weights (re-DMAed per rep; wq sliced on rep 0 for a fast start)
    dma_x(0)
    wq = P["wp"].tile([128, KT * OC], BF16, tag="wq")
    wk = P["wp"].tile([128, KT * OC], BF16, tag="wk")
    wv = P["wp"].tile([128, KT * OC], BF16, tag="wv")
    wg8 = P["wp"].tile([128, KT8 * 2 * OC], FP8, tag="wg8")
    wo = P["wp"].tile([128, HPC * H], BF16, tag="wo")
    if first:
        for kt in range(KT):
            nc.sync.dma_start(wq[:, kt * OC:(kt + 1) * OC],
                              D["wq_d"][:, kt * OC:(kt + 1) * OC])
    else:
        nc.sync.dma_start(wq[:], D["wq_d"][:])
    nc.sync.dma_start(wk[:], D["wk_d"][:])
    nc.sync.dma_start(wv[:], D["wv_d"][:])
    nc.sync.dma_start(wg8[:], D["wg8_d"][:])
    nc.sync.dma_start(wo[:], D["wo_d"][:])
    dma_x(1)

    if first:
        # keep the PE busy during the initial DMA wait so HAM is warm
        pwarm = P["pap"].tile([128, 512], BF16, tag="pa")
        for _w in range(40):
            nc.tensor.transpose(pwarm[:, 0:128], C["ident"][:], C["ident"][:])

    # ---- projection chains (channel-major q/k, token-major v, fp8 gate)
    def qk_chain(c, h, kind):
        wmat = wq if kind == "q" else wk
        key = (kind + "T", c)
        if key not in st:
            st[key] = P["qkp"].tile([128, HPC * BLK], BF16, tag=kind + "T",
                                    name=kind + "T")
        dst = st[key]
        xt = st[("x", c)]
        pq = P["pqp"].tile([128, BLK], F32, tag="pq")
        for kt in range(KT):
            nc.tensor.matmul(
                pq[:],
                lhsT=wmat[:, kt * OC + h * HD: kt * OC + (h + 1) * HD],
                rhs=xt[:, kt * BLK:(kt + 1) * BLK],
                start=(kt == 0), stop=(kt == KT - 1),
            )
        # silu(x) = x * sigmoid(x): keeps ACT inside one act-func table
        # (sigmoid_and_others also covers Copy) - no LoadActFuncSet churn.
        sig = P["sigp"].tile([128, BLK], BF16, tag="sig")
        nc.scalar.activation(sig[:], pq[:], AF.Sigmoid)
        nc.vector.tensor_mul(dst[:, h * BLK:(h + 1) * BLK], pq[:], sig[:])

    def v_chain(c, s):
        xt = st[("x", c)]
        pv = P["pyp"].tile([128, OC], F32, tag="py")
        for kt in range(KT):
            nc.tensor.matmul(
                pv[:],
                lhsT=xt[:, kt * BLK + s * SUB: kt * BLK + s * SUB + SUB],
                rhs=wv[:, kt * OC:(kt + 1) * OC],
                start=(kt == 0), stop=(kt == KT - 1),
            )
        v = P["vp"].tile([128, OC], BF16, tag="vt")
        vsig = P["sigp"].tile([128, OC], BF16, tag="vsig")
        nc.scalar.activation(vsig[:], pv[:], AF.Sigmoid)
        nc.vector.tensor_mul(v[:], pv[:], vsig[:])
        st[("v", c, s)] = v

    def g_chain(c, h):
        x83 = st[("x8", c)][:].rearrange("p (k i t) -> p k i t", k=KT8, i=2)
        wg3 = wg8[:].rearrange("p (k i o) -> p k i o", k=KT8, i=2)
        key = ("gT", c)
        if key not in st:
            st[key] = P["gp"].tile([128, 2 * OC], BF16, tag="gT", name="gT")
        gT = st[key]
        # DoubleRow out must sit at PSUM partitions 0..63 (ISA limit), so the
        # two 64-channel halves land side by side in one [64, 512] bank; the
        # upper half is lane-shifted into gT[64:128] with an SBUF->SBUF DMA.
        pg = P["pqp"].tile([64, 2 * BLK], F32, tag="pq")
        for half in range(2):
            m0 = h * HD + half * 64
            for kt in range(KT8):
                nc.tensor.matmul(
                    pg[:, half * BLK:(half + 1) * BLK],
                    lhsT=wg3[:, kt, :, m0:m0 + 64],
                    rhs=x83[:, kt, :, :],
                    start=(kt == 0), stop=(kt == KT8 - 1),
                    perf_mode=DR,
                )
        gT3 = gT[:].rearrange("p (s ht) -> p s ht", s=2)
        pg_lo = pg[:, 0:BLK].rearrange("p (s t) -> p s t", s=2)
        nc.scalar.activation(gT3[0:64, :, h * SUB:(h + 1) * SUB], pg_lo,
                             AF.Sigmoid, scale=1.0 / SW_G)
        pg_hi = pg[:, BLK:2 * BLK].rearrange("p (s t) -> p s t", s=2)
        ghi = P["sigp"].tile([64, BLK], BF16, tag="ghi")
        ghi3 = ghi[:].rearrange("p (s t) -> p s t", s=2)
        nc.scalar.activation(ghi3, pg_hi, AF.Sigmoid, scale=1.0 / SW_G)
        nc.sync.dma_start(gT3[64:128, :, h * SUB:(h + 1) * SUB], ghi3)

    def qtd_issue(c, s):
        if c == 0 and s == 0:
            st[("qtd", c, s)] = None
            return
        qT3 = st[("qT", c)][:].rearrange("p (h t) -> p h t", h=HPC)
        qd3 = C["qd"][:].rearrange("p (h t) -> p h t", h=HPC)
        qtd = P["qtdp"].tile([128, OC], BF16, tag="qtd")
        qtd3 = qtd[:].rearrange("p (h t) -> p h t", h=HPC)
        nc.gpsimd.tensor_mul(qtd3, qT3[:, :, s * SUB:(s + 1) * SUB], qd3)
        st[("qtd", c, s)] = qtd

    def proj_chunk(c):
        """All 14 projection chains for chunk c, unpipelined (prologue)."""
        for h in range(HPC):
            qk_chain(c, h, "q")
        for h in range(HPC):
            qk_chain(c, h, "k")
        for s in range(2):
            v_chain(c, s)
        for h in range(HPC):
            g_chain(c, h)
        qtd_issue(c, 0)
        qtd_issue(c, 1)

    # ---- attention stages for chunk c, sub-block s
    def attn_front(c, s):
        ts = s * SUB
        kT, qT = st[("kT", c)], st[("qT", c)]
        pqk = P["pap"].tile([128, 4 * SUB], F32, tag="pa")
        for h in range(HPC):
            nc.tensor.matmul(
                pqk[:, h * SUB:(h + 1) * SUB],
                lhsT=kT[:, h * BLK + ts: h * BLK + ts + SUB],
                rhs=qT[:, h * BLK + ts: h * BLK + ts + SUB],
                start=True, stop=True,
            )
        qkm = P["qkmp"].tile([128, 4 * SUB], BF16, tag="qkm")
        nc.vector.tensor_mul(qkm[:], pqk[:], C["dT"][:])

        ptr = P["pap"].tile([128, 4 * SUB], BF16, tag="pa")
        for h in range(HPC):
            nc.tensor.transpose(
                ptr[:, h * SUB:(h + 1) * SUB],
                kT[:, h * BLK + ts: h * BLK + ts + SUB], C["ident"][:])
        ktok = P["ktokp"].tile([128, 4 * SUB], BF16, tag="ktok")
        for h in range(HPC):
            nc.scalar.activation(
                ktok[:, h * SUB:(h + 1) * SUB],
                ptr[:, h * SUB:(h + 1) * SUB],
                AF.Copy, scale=C["kdc"][:, h:h + 1])
        st[("qkm", c, s)] = qkm
        st[("ktok", c, s)] = ktok

    def attn_back(c, s):
        qkm, ktok = st[("qkm", c, s)], st[("ktok", c, s)]
        vt = st[("v", c, s)]
        qtd = st[("qtd", c, s)]
        kv_cur = st.get("kv")
        po = P["pap"].tile([128, 4 * SUB], F32, tag="pa")
        for h in range(HPC):
            nc.tensor.matmul(
                po[:, h * SUB:(h + 1) * SUB],
                lhsT=vt[:, h * HD:(h + 1) * HD],
                rhs=qkm[:, h * SUB:(h + 1) * SUB],
                start=True, stop=(kv_cur is None),
            )
            if kv_cur is not None:
                nc.tensor.matmul(
                    po[:, h * SUB:(h + 1) * SUB],
                    lhsT=kv_cur[:, h * HD:(h + 1) * HD],
                    rhs=qtd[:, h * SUB:(h + 1) * SUB],
                    start=False, stop=True,
                )
        oT = P["otp"].tile([128, 4 * SUB], BF16, tag="oT")
        nc.scalar.activation(oT[:], po[:], AF.Copy)
        sq = P["sqp"].tile([128, 4 * SUB], BF16, tag="sq")
        nc.gpsimd.tensor_mul(sq[:], oT[:], oT[:])
        st[("oT", c, s)] = oT
        st[("sq", c, s)] = sq

        pkv = P["pap"].tile([128, 4 * SUB], F32, tag="pa")
        for h in range(HPC):
            nc.tensor.matmul(
                pkv[:, h * HD:(h + 1) * HD],
                lhsT=ktok[:, h * SUB:(h + 1) * SUB],
                rhs=vt[:, h * HD:(h + 1) * HD],
                start=True, stop=True,
            )
        kv_new = P["kvp"].tile([128, OC], BF16, tag="kv")
        if kv_cur is None:
            nc.vector.tensor_copy(kv_new[:], pkv[:])
        else:
            for h in range(HPC):
                nc.vector.scalar_tensor_tensor(
                    out=kv_new[:, h * HD:(h + 1) * HD],
                    in0=kv_cur[:, h * HD:(h + 1) * HD],
                    scalar=C["bdc"][:, h:h + 1],
                    in1=pkv[:, h * HD:(h + 1) * HD],
                    op0=ALU.mult, op1=ALU.add)
        st["kv"] = kv_new

    def z_issue(c, s):
        z = P["zp"].tile([128, 4 * SUB], BF16, tag="z")
        gT = st[("gT", c)]
        nc.vector.tensor_mul(z[:], st[("oT", c, s)][:],
                             gT[:, s * OC:(s + 1) * OC])
        st[("z", c, s)] = z

    def ssq_chain(c, s):
        sq = st[("sq", c, s)]
        pssq = P["pap"].tile([1, SUB], F32, tag="pa")
        for h in range(HPC):
            nc.tensor.matmul(pssq[:], lhsT=C["ones"][:],
                             rhs=sq[:, h * SUB:(h + 1) * SUB],
                             start=(h == 0), stop=(h == HPC - 1))
        ssq_s = P["ssqp"].tile([1, SUB], F32, tag="ssqs")
        nc.vector.tensor_copy(ssq_s[:], pssq[:])
        t0 = c * BLK
        nc.sync.dma_start(D["ssq_d"][0:1, t0 + s * SUB: t0 + (s + 1) * SUB],
                          ssq_s[:])

    def out_chain(c, s, fb):
        z = st[("z", c, s)]
        py = P["pyp"].tile([128, 512], F32, tag="py")
        for h in range(HPC):
            nc.tensor.matmul(
                py[:],
                lhsT=z[:, h * SUB:(h + 1) * SUB],
                rhs=wo[:, h * H + fb * 512: h * H + (fb + 1) * 512],
                start=(h == 0), stop=(h == HPC - 1),
            )
        y_s = P["ysp"].tile([128, 512], BF16, tag="ys")
        if fb % 2 == 0:
            nc.vector.tensor_copy(y_s[:], py[:])
        else:
            nc.scalar.activation(y_s[:], py[:], AF.Copy)
        t0 = c * BLK
        nc.sync.dma_start(
            D["y_d"][t0 + s * SUB: t0 + (s + 1) * SUB,
                     fb * 512:(fb + 1) * 512],
            y_s[:],
        )

    # ---- prologue: chunk-0 projections, then the pipelined chunk loop.
    proj_chunk(0)

    for c in range(NCHUNK):
        n = c + 1
        piped = n < NCHUNK

        def p(fn, *a):
            if piped:
                fn(n, *a)

        attn_front(c, 0)
        p(qk_chain, 0, "q")
        attn_back(c, 0)
        z_issue(c, 0)
        p(qk_chain, 1, "q")
        attn_front(c, 1)
        p(qk_chain, 2, "q")
        attn_back(c, 1)
        z_issue(c, 1)
        p(qk_chain, 3, "q")
        if piped:
            qtd_issue(n, 0)
            qtd_issue(n, 1)
        p(qk_chain, 0, "k")
        out_chain(c, 0, 0)
        out_chain(c, 0, 1)
        p(qk_chain, 1, "k")
        out_chain(c, 0, 2)
        out_chain(c, 0, 3)
        p(qk_chain, 2, "k")
        p(qk_chain, 3, "k")
        out_chain(c, 1, 0)
        out_chain(c, 1, 1)
        p(v_chain, 0)
        out_chain(c, 1, 2)
        out_chain(c, 1, 3)
        p(v_chain, 1)
        ssq_chain(c, 0)
        ssq_chain(c, 1)
        if piped:
            for h in range(HPC):
                g_chain(n, h)
            dma_x(n + 1)


_NC_CACHE = {}


def get_nc():
    if "nc" not in _NC_CACHE:
        _NC_CACHE["nc"] = build_nc()
    return _NC_CACHE["nc"]


def _prep_core_inputs(hidden_states, slope_rate, w_qkv, w_gate, w_out, norm_w):
    """Returns list of 8 in_map dicts."""
    x = np.asarray(hidden_states, dtype=np.float32)
    slopes = np.asarray(slope_rate, dtype=np.float32).reshape(NH)
    w_qkv = np.asarray(w_qkv, dtype=np.float32)
    w_gate = np.asarray(w_gate, dtype=np.float32)
    w_out = np.asarray(w_out, dtype=np.float32)
    norm_w = np.asarray(norm_w, dtype=np.float32)

    def to_sb(wT):  # [2048, F] -> [128, KT*F] bf16
        f = wT.shape[1]
        return np.ascontiguousarray(
            wT.reshape(KT, 128, f).transpose(1, 0, 2).reshape(128, KT * f)
        ).astype(BF)

    x_sb, x8_sb = [], []
    for b in range(B):
        xT = x[b].T  # [2048 f, 4096 n]
        # [128, NCHUNK, KT*BLK]: [p, c, kt*256+t] = xT[kt*128+p, c*256+t]
        xs = xT.reshape(KT, 128, NCHUNK, BLK).transpose(1, 2, 0, 3)
        xs = np.ascontiguousarray(xs.reshape(128, NCHUNK, KT * BLK))
        x_sb.append(xs.astype(BF))
        x8_sb.append(xs.astype(F8NP))

    arr = np.arange(1, SUB + 1, dtype=np.float32)
    in_maps = []
    for core in range(NCORES):
        b, hg = divmod(core, HPC)
        heads = [HPC * hg + j for j in range(HPC)]
        wq = np.concatenate([w_qkv[h * 384: h * 384 + 128] for h in heads], 0)
        wk = np.concatenate([w_qkv[h * 384 + 128: h * 384 + 256] for h in heads], 0)
        wv = np.concatenate([w_qkv[h * 384 + 256: h * 384 + 384] for h in heads], 0)
        wo = w_out[:, hg * OC:(hg + 1) * OC] * norm_w[None, hg * OC:(hg + 1) * OC]
        # wo: [2048 f, 512 o] -> [128 p(o), 4 h, 2048 f]
        wo_sb = np.ascontiguousarray(
            wo.T.reshape(HPC, 128, H).transpose(1, 0, 2).reshape(128, HPC * H)
        ).astype(BF)
        # gate weights, fp8 e4m3, [128 p, kt8, 2, 512 och] * SW_G
        wgl = w_gate[hg * OC:(hg + 1) * OC]  # [512 o, 2048 f]
        wg8 = np.ascontiguousarray(
            (wgl.T * SW_G).reshape(KT8, 2, 128, OC).transpose(2, 0, 1, 3)
            .reshape(128, KT8 * 2 * OC)).astype(F8NP)

        dTc = np.zeros((128, HPC * SUB), dtype=np.float32)
        qdec = np.zeros((128, HPC * SUB), dtype=np.float32)
        kdcc = np.zeros((128, HPC), dtype=np.float32)
        bdec = np.zeros((128, HPC), dtype=np.float32)
        for j, h in enumerate(heads):
            s = slopes[h]
            idx = arr[:, None] - arr[None, :]  # [m, n]
            full = np.where(idx >= 0, np.exp(-s * np.maximum(idx, 0.0)),
                            0.0).astype(np.float32)
            # decayT[n, m] layout: partition n (k-token), col m (q-token)
            dTc[:, j * SUB:(j + 1) * SUB] = full.T
            kdcc[:, j] = np.exp(-s * (SUB - arr))
            qdec[:, j * SUB:(j + 1) * SUB] = np.exp(-s * arr)[None, :]
            bdec[:, j] = np.exp(-s * np.float32(SUB))

        in_maps.append({
            "ones_sb": np.ones((128, 1), dtype=BF),
            "ident_sb": np.eye(128, dtype=np.float32).astype(BF),
            "x_sb": x_sb[b],
            "x8_sb": x8_sb[b],
            "wq_sb": to_sb(np.ascontiguousarray(wq.T)),
            "wk_sb": to_sb(np.ascontiguousarray(wk.T)),
            "wv_sb": to_sb(np.ascontiguousarray(wv.T)),
            "wg8_sb": wg8,
            "wo_sb": wo_sb,
            "decayT": dTc,
            "qdec": qdec,
            "kdecc": kdcc,
            "bdec": bdec,
        })
    return in_maps


def _assemble(results):
    out = np.zeros((B, N, H), dtype=np.float32)
    for b in range(B):
        ys = [np.asarray(results[HPC * b + g]["y"], dtype=np.float32)
              for g in range(HPC)]
        ssqs = [results[HPC * b + g]["ssq"].reshape(N).astype(np.float32)
                for g in range(HPC)]
        y_sum = ys[0] + ys[1] + ys[2] + ys[3]
        ssq = ssqs[0] + ssqs[1] + ssqs[2] + ssqs[3]
        rfac = 1.0 / np.sqrt(ssq / np.float32(NH * HD) + np.float32(EPS))
        out[b] = y_sum * rfac[:, None].astype(np.float32)
    return out


def kernel(hidden_states, slope_rate, w_qkv, w_gate, w_out, norm_w):
    nc = get_nc()
    in_maps = _prep_core_inputs(hidden_states, slope_rate, w_qkv, w_gate,
                                w_out, norm_w)
    res = run_bass_kernel_spmd(nc, in_maps, core_ids=list(range(NCORES)))
    return _assemble(res.results)
